# revision 8
# baseline (speedup 1.0000x reference)
"""Multi-head causal attention on 8 Trainium2 NeuronCores, fp8-accelerated.

Sharding: core c -> (batch b = c//2, head-half hh = c%2).  Each core computes
q/k/v projections for its 8 heads (column-sharded wq/wk/wv), causal attention,
and a full-width partial output projection (row-sharded wo).  Host sums the
two partials per batch and adds the bias.

fp8 strategy (error budget: harness gate 2e-2, this config sims at 1.4e-2):
the PE runs fp8e4 DoubleRow matmuls (K=256 per pass, 2x MACs/column) wherever
the contraction exceeds 128 and the numerics allow:
  - q/k/v projections for s >= 512 (x8 @ w8, DR over kg-pairs); s < 512 stays
    bf16 to protect the early rows where ctx ~ v has no averaging.
  - attention ctx for off-diagonal j-tile pairs (fp8 exp x fp8 v_ext, one DR
    matmul per head per pair); diagonal tiles keep the bf16 per-tile path
    (mask handling, and the first i-block is all-diagonal).
  - output projection for i >= 512 (fp8 ctx x fp8 wo, DR over e-pairs).
All exps carry a -6*ln2 bias so fp8 exp values stay in [0, ~100] (fp8e4
saturates to inf at 240); the shift cancels in the softmax normalization.
Scores stay bf16: with K=64 the matmul is column-bound and fp8 gains nothing.

Device-side layout trick: scores are computed transposed (scoresT[j, i]) so
that the softmax-weighted sum over keys (ctx) is a plain matmul with v as the
stationary operand.  Ones-columns baked alongside v produce the softmax
denominator replicated across 64 partitions in the same PSUM tile as ctx.

Scheduling: a single interleaved program.  Projections for s-block sb+1 and
the output projection for i-block ib-1 are chopped into small generator
"filler units" issued between the score/ctx matmuls of attention block ib, so
the in-order PE queue always has independent work while the scalar engine
(exp) is the pacer.  Ctx PSUM accumulators are evicted to SBUF immediately
after the last ctx matmul (releasing the PSUM bank); the softmax
normalization (reciprocal rows, DMA partition-broadcast, multiplies) runs off
the critical path on the DVE/GpSimd/DMA engines.
"""

import numpy as np
import ml_dtypes

import concourse.bass as bass
import concourse.mybir as mybir
import concourse.tile as tile
from concourse import bacc
from concourse.bass_utils import run_bass_kernel_spmd

# Problem shape (hardcoded; kernel.py must be self-contained).
B, S, D, H = 4, 2048, 1024, 16
HD = D // H           # 64 head dim
NCORES = 8
EH = D // 2           # 512: per-core e-width (8 heads)
SB = 512              # s-block (free dim of most matmuls)
NSB = S // SB         # 4
NST = S // 128        # 16 s-tiles / j-tiles
NEG = EH // 128       # 4 e-groups of 128 partitions (head pairs)
NKG = D // 128        # 8 d-groups (contraction tiles)
NKP = NKG // 2        # 4 kg-pairs for fp8 DoubleRow
NV8 = 12              # j-tiles with an fp8 v copy (off-diag use only)
VROW = 4 * 192        # v_ext row: 4x [v_odd(64) | ones(64) | v_even(64)]

F32 = mybir.dt.float32
BF16 = mybir.dt.bfloat16
FP8 = mybir.dt.float8e4
DRM = mybir.MatmulPerfMode.DoubleRow
MMDT = BF16
MMNP = ml_dtypes.bfloat16
NPF8 = ml_dtypes.float8_e4m3
EXP_BIAS = float(-6.0 * np.log(2.0))  # keeps fp8 exp under ~100 (inf at 240)

TRACE = False
LAST_RESULT = None


def _build():
    nc = bacc.Bacc()

    xT_d = nc.dram_tensor("xt", [D, SB], MMDT, kind="ExternalInput")
    x8_d = nc.dram_tensor("xt8", [D, 3 * SB], FP8, kind="ExternalInput")
    wqT_d = nc.dram_tensor("wqt", [D, EH], MMDT, kind="ExternalInput")
    wkT_d = nc.dram_tensor("wkt", [D, EH], MMDT, kind="ExternalInput")
    wvT_d = nc.dram_tensor("wvt", [D, EH], MMDT, kind="ExternalInput")
    wq8_d = nc.dram_tensor("wqt8", [D, EH], FP8, kind="ExternalInput")
    wk8_d = nc.dram_tensor("wkt8", [D, EH], FP8, kind="ExternalInput")
    wv8_d = nc.dram_tensor("wvt8", [D, EH], FP8, kind="ExternalInput")
    woT_d = nc.dram_tensor("wot", [EH, D], MMDT, kind="ExternalInput")
    wo8_d = nc.dram_tensor("wot8", [EH, D], FP8, kind="ExternalInput")
    masks_d = nc.dram_tensor("masks", [128, 128], MMDT, kind="ExternalInput")
    out_d = nc.dram_tensor("out", [S, D], MMDT, kind="ExternalOutput")
    scr_d = nc.dram_tensor("dscr", [4, NSB, 2, SB], F32)

    with tile.TileContext(nc) as tc:
        with (
            tc.tile_pool(name="persist", bufs=1) as persist,
            tc.tile_pool(name="wts", bufs=1) as wts,
            tc.tile_pool(name="xp", bufs=1) as xp,
            tc.tile_pool(name="xp8", bufs=2) as xp8,
            tc.tile_pool(name="expp", bufs=4) as expp,
            tc.tile_pool(name="expp8", bufs=3) as expp8,
            tc.tile_pool(name="crawp", bufs=2) as crawp,
            tc.tile_pool(name="rrp", bufs=2) as rrp,
            tc.tile_pool(name="bcp", bufs=2) as bcp,
            tc.tile_pool(name="otp", bufs=2) as otp,
            tc.tile_pool(name="sp", bufs=2, space="PSUM") as sp,
            tc.tile_pool(name="pcp", bufs=2, space="PSUM") as pcp,
            tc.tile_pool(name="accp", bufs=2, space="PSUM") as accp,
        ):
            qT = persist.tile([128, NEG, S], MMDT)      # [e-part, e-group, s]
            kT = persist.tile([128, NEG, S], MMDT)
            v_ext = persist.tile([128, NST, VROW], MMDT)  # [s-part, s-tile, row]
            v8_ext = persist.tile([128, NV8, VROW], FP8)
            ctxT = persist.tile([128, NEG, SB], MMDT)       # i in [0, 512)
            ctxT8 = persist.tile([128, NEG, 3 * SB], FP8)   # i in [512, 2048)

            masks_sb = wts.tile([128, 128], MMDT)
            w_q = wts.tile([128, NKG, EH], MMDT)
            w_k = wts.tile([128, NKG, EH], MMDT)
            w_v = wts.tile([128, NKG, EH], MMDT)
            w_q8 = wts.tile([128, NKP, 2, EH], FP8)
            w_k8 = wts.tile([128, NKP, 2, EH], FP8)
            w_v8 = wts.tile([128, NKP, 2, EH], FP8)
            woT_sb = wts.tile([128, NEG, D], MMDT)
            wo8_sb = wts.tile([128, NEG, D], FP8)
            biasc = wts.tile([128, 1], F32)

            # ---- weight preloads split across the two DMA-issuing queues
            # (gpsimd: w_q then w_v; sync gets x(sb=0) first, then w_k) so
            # the first q/k/v projection blocks are fed as early as possible
            for kg in range(NKG):
                sl = slice(kg * 128, (kg + 1) * 128)
                nc.gpsimd.dma_start(out=w_q[:, kg, :], in_=wqT_d[sl, :])
            for kg in range(0, NKG, 2):
                sl = slice(kg * 128, (kg + 1) * 128)
                nc.gpsimd.dma_start(out=w_v[:, kg, :], in_=wvT_d[sl, :])
            for gg in range(NEG):
                nc.gpsimd.dma_start(
                    out=woT_sb[:, gg, :], in_=woT_d[gg * 128 : (gg + 1) * 128, :]
                )

            nc.vector.memset(biasc, EXP_BIAS)
            # shared ones block between each (even, odd) head pair; a single
            # strided memset covers all 64 blocks
            v_flat = v_ext[:].rearrange("p st (a w) -> p (st a) w", w=192)
            nc.vector.memset(v_flat[:, :, 64:128], 1.0)
            v8_flat = v8_ext[:].rearrange("p st (a w) -> p (st a) w", w=192)
            nc.vector.memset(v8_flat[:, :, 64:128], 1.0)
            # f32 ones column block for the PE-broadcast normalize route
            onesb = wts.tile([128, 64], MMDT)
            nc.vector.memset(onesb, 1.0)

            xts = {}
            xts8 = {}

            def dma_x0():
                t = xp.tile([128, NKG, SB], MMDT, tag="xts")
                for kg in range(NKG):
                    nc.sync.dma_start(
                        out=t[:, kg, :],
                        in_=xT_d[kg * 128 : (kg + 1) * 128, :],
                    )
                xts[0] = t

            def dma_x8(sb):
                t = xp8.tile([128, NKP, 2, SB], FP8, tag="xts8")
                csl = slice((sb - 1) * SB, sb * SB)
                for kg in range(NKG):
                    nc.sync.dma_start(
                        out=t[:, kg // 2, kg % 2, :],
                        in_=x8_d[kg * 128 : (kg + 1) * 128, csl],
                    )
                xts8[sb] = t

            # ---- filler generators: each yield = ~2 matmuls of issued work ----
            def gen_p1_qk(sb, w_sb, dst, mt):
                ssl = slice(sb * SB, (sb + 1) * SB)
                msl = slice(mt * 128, (mt + 1) * 128)
                x_t = xts[sb]
                ps = accp.tile([128, SB], F32, tag="acc")
                for kg in range(NKG):
                    nc.tensor.matmul(
                        out=ps,
                        lhsT=w_sb[:, kg, msl],
                        rhs=x_t[:, kg, :],
                        start=(kg == 0),
                        stop=(kg == NKG - 1),
                    )
                    if kg % 2 == 1:
                        yield
                nc.vector.tensor_copy(dst[:, mt, ssl], ps)
                yield

            def gen_p1_qk8(sb, w8, dst, mt):
                ssl = slice(sb * SB, (sb + 1) * SB)
                msl = slice(mt * 128, (mt + 1) * 128)
                x_t = xts8[sb]
                ps = accp.tile([128, SB], F32, tag="acc")
                for kp in range(NKP):
                    nc.tensor.matmul(
                        out=ps,
                        lhsT=w8[:, kp, :, msl],
                        rhs=x_t[:, kp, :, :],
                        start=(kp == 0),
                        stop=(kp == NKP - 1),
                        perf_mode=DRM,
                    )
                    if kp % 2 == 1:
                        yield
                nc.vector.tensor_copy(dst[:, mt, ssl], ps)
                yield

            def _v_evict(ps, st):
                # psum cols: head h at [h*64, h*64+64); dest pair p:
                # even head -> p*192+128, odd head -> p*192
                psr = ps[:].rearrange("p (a c) -> p a c", c=128)
                vst = v_ext[:, st, :].rearrange("p (a w) -> p a w", w=192)
                nc.vector.tensor_copy(vst[:, :, 128:192], psr[:, :, 0:64])
                nc.vector.tensor_copy(vst[:, :, 0:64], psr[:, :, 64:128])
                if st < NV8:
                    # (gpsimd cannot read PSUM; these ride the DVE queue)
                    v8st = v8_ext[:, st, :].rearrange("p (a w) -> p a w", w=192)
                    nc.vector.tensor_copy(v8st[:, :, 128:192], psr[:, :, 0:64])
                    nc.vector.tensor_copy(v8st[:, :, 0:64], psr[:, :, 64:128])

            def gen_p1_v(sb, st4):
                st = sb * 4 + st4
                x_t = xts[sb]
                xsl = slice(st4 * 128, (st4 + 1) * 128)
                ps = accp.tile([128, EH], F32, tag="acc")
                for kg in range(NKG):
                    nc.tensor.matmul(
                        out=ps,
                        lhsT=x_t[:, kg, xsl],
                        rhs=w_v[:, kg, :],
                        start=(kg == 0),
                        stop=(kg == NKG - 1),
                    )
                    if kg % 2 == 1:
                        yield
                _v_evict(ps, st)
                yield

            def gen_p1_v8(sb, st4):
                st = sb * 4 + st4
                x_t = xts8[sb]
                xsl = slice(st4 * 128, (st4 + 1) * 128)
                ps = accp.tile([128, EH], F32, tag="acc")
                for kp in range(NKP):
                    nc.tensor.matmul(
                        out=ps,
                        lhsT=x_t[:, kp, :, xsl],
                        rhs=w_v8[:, kp, :, :],
                        start=(kp == 0),
                        stop=(kp == NKP - 1),
                        perf_mode=DRM,
                    )
                    if kp % 2 == 1:
                        yield
                _v_evict(ps, st)
                yield

            def gen_p3(ib, it, ob):
                itsl = slice(it * 128, (it + 1) * 128)
                osl = slice(ob * SB, (ob + 1) * SB)
                if ib == NSB - 1 and (it + ob) % 2 == 0:
                    # scores pool is idle during the final drain: borrow its
                    # banks so four accumulators rotate instead of two
                    ps_wide = sp.tile([128, 2 * SB], F32, tag="s", name="p3s")
                    ps = ps_wide[:, 0:SB]
                else:
                    ps = accp.tile([128, SB], F32, tag="acc")
                if ib == 0:
                    for gg in range(NEG):
                        nc.tensor.matmul(
                            out=ps,
                            lhsT=ctxT[:, gg, itsl],
                            rhs=woT_sb[:, gg, osl],
                            start=(gg == 0),
                            stop=(gg == NEG - 1),
                        )
                        if gg % 2 == 1:
                            yield
                else:
                    it8sl = slice(it * 128 - SB, (it + 1) * 128 - SB)
                    for gp in (0, 2):
                        nc.tensor.matmul(
                            out=ps,
                            lhsT=ctxT8[:, gp : gp + 2, it8sl],
                            rhs=wo8_sb[:, gp : gp + 2, osl],
                            start=(gp == 0),
                            stop=(gp == 2),
                            perf_mode=DRM,
                        )
                        if gp == 2:
                            yield
                ot = otp.tile([128, SB], MMDT, tag="ot")
                if ib == NSB - 1:
                    # attention is over: the scalar engine is idle, and the
                    # vector queue would otherwise stall PSUM recycling here
                    nc.scalar.copy(ot, ps)
                    # alternate the final out-DMAs across both issuing
                    # queues so the drain is not serialized on one ring
                    eng = nc.sync if (it + ob) % 2 == 0 else nc.gpsimd
                    eng.dma_start(out=out_d[itsl, osl], in_=ot)
                else:
                    nc.vector.tensor_copy(ot, ps)
                    nc.sync.dma_start(out=out_d[itsl, osl], in_=ot)
                yield

            def p1_gens(sb, mts=range(NEG)):
                gens = []
                for mt in mts:
                    gens.append(gen_p1_qk(sb, w_q, qT, mt))
                    gens.append(gen_p1_qk(sb, w_k, kT, mt))
                return gens

            def p18_gens(sb, mts=range(NEG)):
                gens = []
                for mt in mts:
                    gens.append(gen_p1_qk8(sb, w_q8, qT, mt))
                    gens.append(gen_p1_qk8(sb, w_k8, kT, mt))
                return gens

            def p1_v_gens(sb):
                return [gen_p1_v(sb, st4) for st4 in range(4)]

            def p1_v8_gens(sb):
                return [gen_p1_v8(sb, st4) for st4 in range(4)]

            def p3_gens(ib, its=None):
                if its is None:
                    its = range(4 * ib, 4 * ib + 4)
                return [gen_p3(ib, it, ob) for it in its for ob in range(2)]

            def run_all(gens):
                for g in gens:
                    for _ in g:
                        pass

            # ---- attention ----
            def scores(ib, pr, jt):
                r = jt - 4 * ib
                f0 = 128 * r if r > 0 else 0
                jsl = slice(jt * 128, (jt + 1) * 128)
                qsl = slice(ib * SB + f0, (ib + 1) * SB)
                ps_s = sp.tile([128, 2 * SB], F32, tag="s")
                nc.tensor.matmul(
                    out=ps_s[:, f0:SB],
                    lhsT=kT[0:64, pr, jsl],
                    rhs=qT[0:64, pr, qsl],
                    start=True,
                    stop=True,
                )
                nc.tensor.matmul(
                    out=ps_s[:, SB + f0 : 2 * SB],
                    lhsT=kT[64:128, pr, jsl],
                    rhs=qT[64:128, pr, qsl],
                    start=True,
                    stop=True,
                )
                return ps_s

            def softmax_ctx(ib, pr, jt, ps_s, ps_c0, ps_c1, njt, split=False):
                r = jt - 4 * ib
                f0 = 128 * r if r > 0 else 0
                expT = expp.tile([128, 2 * SB], MMDT, tag="exp")
                ps_v = ps_s[:].rearrange("p (t c) -> p t c", t=2)
                ex_v = expT[:].rearrange("p (t c) -> p t c", t=2)
                if not split:
                    nc.scalar.activation(
                        out=ex_v[:, :, f0:SB],
                        in_=ps_v[:, :, f0:SB],
                        func=mybir.ActivationFunctionType.Exp,
                        scale=1.0 / np.sqrt(HD),
                        bias=biasc[:, :],
                    )
                    if r >= 0:
                        nc.vector.tensor_mul(
                            ex_v[:, :, f0 : f0 + 128],
                            ex_v[:, :, f0 : f0 + 128],
                            masks_sb[:].unsqueeze(1).broadcast_to((128, 2, 128)),
                        )
                for t, ps_c in ((0, ps_c0), (1, ps_c1)):
                    if split:
                        # per-head exp halves the exp->ctx dependency latency
                        nc.scalar.activation(
                            out=ex_v[:, t, f0:SB],
                            in_=ps_v[:, t, f0:SB],
                            func=mybir.ActivationFunctionType.Exp,
                            scale=1.0 / np.sqrt(HD),
                            bias=biasc[:, :],
                        )
                        if r >= 0:
                            nc.vector.tensor_mul(
                                ex_v[:, t, f0 : f0 + 128],
                                ex_v[:, t, f0 : f0 + 128],
                                masks_sb[:, :],
                            )
                    coff = pr * 192 + (64 if t == 0 else 0)
                    nc.tensor.matmul(
                        out=ps_c[:, f0:SB],
                        lhsT=v_ext[:, jt, coff : coff + 128],
                        rhs=expT[:, t * SB + f0 : (t + 1) * SB],
                        start=(jt == 0),
                        stop=(jt == njt - 1),
                    )

            norm_state = {"pending": None}

            def attn(ib, fillers, n_units):
                njt = 4 * (ib + 1)
                isl = slice(ib * SB, (ib + 1) * SB)
                isl8 = slice(ib * SB - SB, (ib + 1) * SB - SB)
                npair = 2 * ib
                slots = 4 * (3 * npair + 4 + 1)
                rate = n_units / slots if slots else 0.0
                queue = list(fillers)
                state = {"acc": 0.0}

                def drive():
                    state["acc"] += rate
                    while state["acc"] >= 1.0 and queue:
                        try:
                            next(queue[0])
                            state["acc"] -= 1.0
                        except StopIteration:
                            queue.pop(0)

                def norm_partA(pr, ps_c0, ps_c1):
                    """Evict PSUM, take local reciprocals, and launch the
                    DRAM-round-trip partition broadcasts.  Returns a closure
                    with the DMA-dependent tail (odd-head reciprocal + gpsimd
                    multiplies), deferred into the next pr's pipeline: by
                    then the broadcasts have landed, so no in-order queue
                    ever stalls on them."""
                    craw = crawp.tile([128, 2, SB], F32, tag="craw")
                    rr = rrp.tile([128, 2, SB], F32, tag="rr")
                    # even-head reciprocal of the partition-0 denominator row
                    # (custom-DVE ops misbehave off partition 0)
                    nc.vector.reciprocal_approx_fast(
                        rr[0:1, 0, :], ps_c0[0:1, :]
                    )
                    nc.vector.tensor_copy(craw[:, 0, :], ps_c0)
                    nc.vector.tensor_copy(craw[:, 1, :], ps_c1)
                    nc.sync.dma_start(
                        out=scr_d[pr, ib, 0, :], in_=rr[0:1, 0, :]
                    )
                    # odd-head raw denominator row (DMA cannot read PSUM)
                    nc.sync.dma_start(
                        out=scr_d[pr, ib, 1, :], in_=craw[64:65, 1, :]
                    )
                    bc = bcp.tile([128, SB], F32, tag="bc")
                    se = scr_d[pr, ib, 0, :]
                    nc.sync.dma_start(
                        out=bc[64:128, :],
                        in_=bass.AP(
                            tensor=se.tensor, offset=se.offset,
                            ap=[[0, 64], [1, SB]],
                        ),
                    )
                    so = scr_d[pr, ib, 1, :]
                    nc.sync.dma_start(
                        out=bc[0:64, :],
                        in_=bass.AP(
                            tensor=so.tensor, offset=so.offset,
                            ap=[[0, 64], [1, SB]],
                        ),
                    )

                    def partB():
                        nc.vector.reciprocal_approx_fast(
                            rr[0:64, 1, :], bc[0:64, :]
                        )
                        # normalize multiplies on gpsimd (all-SBUF operands):
                        # the DVE queue never waits on the broadcast DMAs
                        if ib == 0:
                            d_ev = ctxT[64:128, pr, isl]
                            d_od = ctxT[0:64, pr, isl]
                        else:
                            d_ev = ctxT8[64:128, pr, isl8]
                            d_od = ctxT8[0:64, pr, isl8]
                        nc.gpsimd.tensor_mul(
                            d_ev, craw[64:128, 0, :], bc[64:128, :],
                        )
                        nc.gpsimd.tensor_mul(
                            d_od, craw[0:64, 1, :], rr[0:64, 1, :]
                        )

                    return partB

                def norm_endgame(pr, ps_c0, ps_c1):
                    # Last normalize gates the final output projection: skip
                    # the DRAM round trip entirely via 1-row PE broadcasts
                    # (the PE is idle here), evictions on the idle scalar.
                    craw = crawp.tile([128, 2, SB], F32, tag="craw")
                    rowb = rrp.tile([128, 2, SB], MMDT, tag="rowb")
                    rr = rrp.tile([128, 2, SB], F32, tag="rr")
                    nc.vector.reciprocal_approx_fast(
                        rr[0:1, 0, :], ps_c0[0:1, :]
                    )
                    nc.vector.tensor_copy(rowb[0:1, 0, :], rr[0:1, 0, :])
                    nc.scalar.copy(rowb[64:65, 1, :], ps_c1[64:65, :])
                    nc.scalar.copy(craw[:, 0, :], ps_c0)
                    nc.scalar.copy(craw[:, 1, :], ps_c1)
                    ps_b = accp.tile([128, SB], F32, tag="acc")
                    nc.tensor.matmul(
                        out=ps_b[64:128, :],
                        lhsT=onesb[0:1, :],
                        rhs=rowb[0:1, 0, :],
                        start=True,
                        stop=True,
                    )
                    ps_o = accp.tile([128, SB], F32, tag="acc")
                    nc.tensor.matmul(
                        out=ps_o[0:64, :],
                        lhsT=onesb[64:65, :],
                        rhs=rowb[64:65, 1, :],
                        start=True,
                        stop=True,
                    )
                    nc.vector.reciprocal_approx_fast(
                        rr[0:64, 1, :], ps_o[0:64, :]
                    )
                    nc.vector.tensor_mul(
                        ctxT8[64:128, pr, isl8], craw[64:128, 0, :],
                        ps_b[64:128, :],
                    )
                    nc.vector.tensor_mul(
                        ctxT8[0:64, pr, isl8], craw[0:64, 1, :], rr[0:64, 1, :]
                    )

                for pr in range(4):
                    ps_c0 = pcp.tile([128, SB], F32, tag="pc")
                    ps_c1 = pcp.tile([128, SB], F32, tag="pc")
                    # off-diagonal j-tile pairs: fp8 exp + DoubleRow ctx
                    for jp in range(npair):
                        jt0 = 2 * jp
                        ex8 = expp8.tile([128, 2, 2, SB], FP8, tag="ex8")
                        for s in (0, 1):
                            ps_s = scores(ib, pr, jt0 + s)
                            drive()
                            ps_v = ps_s[:].rearrange("p (t c) -> p t c", t=2)
                            nc.scalar.activation(
                                out=ex8[:, s, :, :],
                                in_=ps_v[:, :, :],
                                func=mybir.ActivationFunctionType.Exp,
                                scale=1.0 / np.sqrt(HD),
                                bias=biasc[:, :],
                            )
                        for t, ps_c in ((0, ps_c0), (1, ps_c1)):
                            coff = pr * 192 + (64 if t == 0 else 0)
                            nc.tensor.matmul(
                                out=ps_c,
                                lhsT=v8_ext[:, jt0 : jt0 + 2, coff : coff + 128],
                                rhs=ex8[:, :, t, :],
                                start=(jp == 0),
                                stop=False,
                                perf_mode=DRM,
                            )
                        drive()
                        if jp == 1 and norm_state["pending"] is not None:
                            norm_state["pending"]()
                            norm_state["pending"] = None
                    # diagonal j-tiles: bf16 per-tile path with masks
                    prev = None
                    for r4 in range(4):
                        jt = 4 * ib + r4
                        ps_prev = prev
                        prev = (jt, scores(ib, pr, jt))
                        drive()
                        if ps_prev is not None:
                            softmax_ctx(ib, pr, *ps_prev, ps_c0, ps_c1, njt,
                                        split=(ib == 0))
                        if ib == 0 and jt == 3 and norm_state["pending"] is not None:
                            norm_state["pending"]()
                            norm_state["pending"] = None
                    drive()
                    softmax_ctx(ib, pr, *prev, ps_c0, ps_c1, njt,
                                split=(ib == 0))
                    if ib == NSB - 1 and pr == 3:
                        norm_endgame(pr, ps_c0, ps_c1)
                    else:
                        norm_state["pending"] = norm_partA(pr, ps_c0, ps_c1)

                # drain leftover fillers
                run_all(queue)

            # ---- main schedule ----
            nc.sync.dma_start(out=masks_sb, in_=masks_d[:, :])
            dma_x0()
            # odd-kg halves of w_v on the sync queue: the v-projection
            # prefix blocks get their full weight set ~4us earlier
            for kg in range(1, NKG, 2):
                sl = slice(kg * 128, (kg + 1) * 128)
                nc.sync.dma_start(out=w_v[:, kg, :], in_=wvT_d[sl, :])
            for kg in range(NKG):
                sl = slice(kg * 128, (kg + 1) * 128)
                nc.sync.dma_start(out=w_k[:, kg, :], in_=wkT_d[sl, :])
            # fp8 weights: needed from the sb=1 projection fillers (attn 0)
            for kg in range(NKG):
                sl = slice(kg * 128, (kg + 1) * 128)
                nc.gpsimd.dma_start(out=w_q8[:, kg // 2, kg % 2, :], in_=wq8_d[sl, :])
                nc.sync.dma_start(out=w_k8[:, kg // 2, kg % 2, :], in_=wk8_d[sl, :])
                nc.gpsimd.dma_start(out=w_v8[:, kg // 2, kg % 2, :], in_=wv8_d[sl, :])
            for gg in range(NEG):
                nc.gpsimd.dma_start(
                    out=wo8_sb[:, gg, :], in_=wo8_d[gg * 128 : (gg + 1) * 128, :]
                )
            # Dummy matmuls gated only on the tiny masks DMA: they execute
            # during the DMA head and ramp the PE clock out of its low power
            # state before the real work arrives.  Results are never read.
            for i in range(40):
                ps_dum = accp.tile([128, 128], F32, tag="acc")
                nc.tensor.matmul(
                    out=ps_dum,
                    lhsT=masks_sb[0:1, :],
                    rhs=masks_sb[0:1, :],
                    start=True,
                    stop=True,
                )
            # minimal prefix of projections so attention(0, pr=0) can start;
            # ordered to match DMA arrival: w_q first (gpsimd), w_v right
            # behind it, w_k last (sync queue, behind masks + x block 0)
            run_all(
                [gen_p1_qk(0, w_q, qT, 0)]
                + p1_v_gens(0)
                + [gen_p1_qk(0, w_k, kT, 0)]
            )
            dma_x8(1)
            for sbi in range(NSB):
                fillers = []
                n_units = 0
                if sbi == 0:
                    fillers += p1_gens(0, mts=[1, 2, 3])
                    n_units += 30
                if sbi == NSB - 1:
                    # k(sb=3) blocks for head pairs 1..3, deferred here from
                    # attn(2): attention only reads them from j-tile 12 of
                    # the matching pr, so they fill this scalar-paced block.
                    fillers += [
                        gen_p1_qk8(sbi, w_k8, kT, mt) for mt in (1, 2, 3)
                    ]
                    n_units += 9
                if sbi < NSB - 1:
                    if sbi > 0:
                        dma_x8(sbi + 1)
                    if sbi == NSB - 2:
                        fillers += [
                            gen_p1_qk8(sbi + 1, w_q8, qT, mt) for mt in range(NEG)
                        ]
                        fillers += [gen_p1_qk8(sbi + 1, w_k8, kT, 0)]
                        fillers += p1_v8_gens(sbi + 1)
                        n_units += 27
                    else:
                        fillers += p18_gens(sbi + 1) + p1_v8_gens(sbi + 1)
                        n_units += 36
                if sbi >= 1:
                    fillers += p3_gens(sbi - 1)
                    n_units += 24 if sbi == 1 else 16
                attn(sbi, fillers, n_units)
            run_all(p3_gens(NSB - 1))

    nc.finalize()
    return nc


_NC = None


def _get_nc():
    global _NC
    if _NC is None:
        _NC = _build()
    return _NC


def kernel(x, wq, wk, wv, wo, wo_b):
    global LAST_RESULT
    x = np.ascontiguousarray(np.asarray(x, dtype=np.float32))
    wq = np.asarray(wq, dtype=np.float32)
    wk = np.asarray(wk, dtype=np.float32)
    wv = np.asarray(wv, dtype=np.float32)
    wo = np.asarray(wo, dtype=np.float32)
    wo_b = np.asarray(wo_b, dtype=np.float32)

    pp, ff = np.ogrid[0:128, 0:128]
    masks = (pp <= ff).astype(np.float32)

    def swiz(w):
        return w.reshape(4, 2, 64, D)[:, ::-1].reshape(EH, D)

    in_maps = []
    for c in range(NCORES):
        b, hh = c // 2, c % 2
        es = slice(hh * EH, (hh + 1) * EH)
        xT = x[b].T
        woTs = np.ascontiguousarray(wo[:, es].T)
        in_maps.append(
            {
                "xt": np.ascontiguousarray(xT[:, 0:SB].astype(MMNP)),
                "xt8": np.ascontiguousarray(xT[:, SB:].astype(NPF8)),
                "wqt": np.ascontiguousarray(wq[es, :].T.astype(MMNP)),
                "wkt": np.ascontiguousarray(wk[es, :].T.astype(MMNP)),
                "wvt": np.ascontiguousarray(wv[es, :].T.astype(MMNP)),
                "wqt8": np.ascontiguousarray(wq[es, :].T.astype(NPF8)),
                "wkt8": np.ascontiguousarray(wk[es, :].T.astype(NPF8)),
                "wvt8": np.ascontiguousarray(wv[es, :].T.astype(NPF8)),
                "wot": np.ascontiguousarray(swiz(woTs).astype(MMNP)),
                "wot8": np.ascontiguousarray(swiz(woTs).astype(NPF8)),
                "masks": masks.astype(MMNP),
            }
        )

    nc = _get_nc()
    res = run_bass_kernel_spmd(nc, in_maps, list(range(NCORES)), trace=TRACE)
    LAST_RESULT = res

    out = np.empty((B, S, D), np.float32)
    for b in range(B):
        out[b] = np.asarray(res.results[2 * b]["out"], np.float32) + np.asarray(res.results[2 * b + 1]["out"], np.float32)
    out += wo_b[None, None, :]
    return out
